# revision 18
# baseline (speedup 1.0000x reference)
"""Trainium2 Bass kernel for nn_AttentionNetwork (ragged path attention).

Data-parallel over 8 NeuronCores: 512 paths per core. Paths are sorted by
length (host-side) and packed into variable-width blocks (bp paths x cap
node-slots, bp*cap <= 1024, cap = max length in the block; capacities are
taken as the element-wise max over cores so one SPMD program serves all
8; the last ~64 paths go into <=256-row blocks so the pipeline drains
quickly). All matmuls run in bf16. Padding slots are filled host-side
with a "poison" vector whose MLP score is ~-3e4, so exp underflows to
exactly 0 and no mask machinery is needed on the device. Per block and
~512-column chunk: node-MLP into paired PSUM banks (one fused relu per
bank pair when b1==0, saving ACT instruction overhead) -> score matmuls
-> exp row; the exp row is broadcast across partitions (GpSimd) into
slot KC of a [128, KC+1, rows] tile whose first KC slots get x*e (one
DVE multiply); segment sums (weighted feature sums + the softmax
denominator in slot KC) are computed by a pairwise fold tree while the
segment width stays even (TensorTensor adds beat TensorReduce on the
DVE) with a TensorReduce tail. Stage 2 (path-level attention) is split
into two path ranges interleaved into the tail of stage 1 so the PE
never drains; its softmax skips max-subtraction (logits are O(1)) and
the 8 cores' exp-weighted partial sums are combined on the host.
"""

import sys

if "/opt/trn_rl_repo" not in sys.path:
    sys.path.insert(0, "/opt/trn_rl_repo")

from contextlib import ExitStack

import ml_dtypes
import numpy as np

import concourse.bass as bass  # noqa: F401
import concourse.mybir as mybir
import concourse.tile as tile
from concourse import bacc, bass_utils

P, LMAX, D, H = 4096, 64, 512, 512
NCORES = 8
PS = P // NCORES          # paths per core
KC = D // 128             # contraction chunks
HC = H // 128             # hidden tiles
ROWS_TARGET = 1024        # max rows (bp*cap) per block
TAIL_PATHS = 128          # last paths go into small blocks ...
TAIL_ROWS = 256           # ... of <=256 rows for a fast pipeline drain

f32 = mybir.dt.float32
bf16 = mybir.dt.bfloat16
AF = mybir.ActivationFunctionType
ALU = mybir.AluOpType
AX = mybir.AxisListType

LAST_RESULT = None
_PROG_CACHE = {}
_TRACE_KW = {}


def _make_blocks(len_max):
    """Greedy pack sorted-desc lengths into (bp, cap) blocks.

    cap and bp are kept even so every matmul free dim / path offset is even.
    """
    blocks = []
    i = 0
    while i < PS:
        cap = int(len_max[i])
        if cap % 2:
            cap += 1
        target = TAIL_ROWS if i >= PS - TAIL_PATHS else ROWS_TARGET
        bp = min(max(target // cap, 2), PS - i)
        if bp % 2 and bp > 1:
            bp -= 1
        blocks.append((bp, cap))
        i += bp
    return tuple(blocks)


def _build_program(blocks, b1_zero, ab1_zero):
    """blocks: tuple of (bp, cap); one block = bp paths x cap node slots."""
    nb = len(blocks)
    rows_list = [bp * cap for bp, cap in blocks]
    tot_rows = sum(rows_list)

    nc = bacc.Bacc("TRN2", target_bir_lowering=False, debug=False, num_devices=NCORES)

    xb = nc.dram_tensor("xb", [KC * 128 * tot_rows], bf16, kind="ExternalInput")
    w1 = nc.dram_tensor("w1", [KC, 128, H], bf16, kind="ExternalInput")
    w2 = nc.dram_tensor("w2", [128, HC], bf16, kind="ExternalInput")
    b1 = nc.dram_tensor("b1", [128, HC], f32, kind="ExternalInput")
    aw1 = nc.dram_tensor("aw1", [KC, 128, H], bf16, kind="ExternalInput")
    ab1 = nc.dram_tensor("ab1", [128, HC], f32, kind="ExternalInput")
    aw2 = nc.dram_tensor("aw2", [128, HC], bf16, kind="ExternalInput")
    out_part = nc.dram_tensor("out_part", [3, 128, KC], f32, kind="ExternalOutput")
    out_stats = nc.dram_tensor("out_stats", [1, 3], f32, kind="ExternalOutput")

    x_offs = [0] * nb
    p_offs = [0] * nb
    acc_x = acc_p = 0
    for i in range(nb):
        x_offs[i], p_offs[i] = acc_x, acc_p
        acc_x += KC * 128 * rows_list[i]
        acc_p += blocks[i][0]
    assert acc_p == PS

    with ExitStack() as ctx:
        tc = ctx.enter_context(tile.TileContext(nc))
        const = ctx.enter_context(tc.tile_pool(name="const", bufs=1))
        xpool = ctx.enter_context(tc.tile_pool(name="x", bufs=6))
        xapool = ctx.enter_context(tc.tile_pool(name="xa", bufs=2))
        hpool = ctx.enter_context(tc.tile_pool(name="h", bufs=2))
        spool = ctx.enter_context(tc.tile_pool(name="s", bufs=3))
        ph_pool = ctx.enter_context(tc.tile_pool(name="ph", bufs=3, space="PSUM"))
        ps_pool = ctx.enter_context(tc.tile_pool(name="ps", bufs=2, space="PSUM"))

        t_w1 = const.tile([128, KC, H], bf16)
        nc.sync.dma_start(t_w1[:], w1.ap().rearrange("k d h -> d k h"))
        t_w2 = const.tile([128, HC], bf16)
        nc.sync.dma_start(t_w2[:], w2.ap())
        t_b1 = const.tile([128, HC], f32)
        nc.sync.dma_start(t_b1[:], b1.ap())
        # ACT table prefetch: force the exp table load before data arrives
        t_warm = const.tile([1, 1], f32)
        nc.scalar.activation(t_warm[:], t_b1[0:1, 0:1], AF.Exp)
        t_aw1 = const.tile([128, KC, H], bf16)
        t_ab1 = const.tile([128, HC], f32)
        t_aw2 = const.tile([128, HC], bf16)

        pfT = const.tile([128, KC, PS], bf16)  # normalized path features

        def mlp(dst, rhs_src, cs, n, wtile, btile, bzero, tag):
            """dst[:, j, cs] = relu(W_j.T @ rhs + b_j) for j in 0..HC-1."""
            for jj in range(HC // 2):
                # full-bank tile so each [:, j2, :] slice is bank-aligned
                ph = ph_pool.tile([128, 2, 512], f32, tag="h", name=f"ph_{tag}_{jj}")
                for j2 in range(2):
                    j = 2 * jj + j2
                    for k in range(KC):
                        nc.tensor.matmul(
                            ph[:, j2, 0:n],
                            wtile[:, k, 128 * j : 128 * (j + 1)],
                            rhs_src(k),
                            start=(k == 0),
                            stop=(k == KC - 1),
                        )
                if bzero:
                    nc.scalar.activation(
                        dst[:, 2 * jj : 2 * jj + 2, cs], ph[:, :, 0:n], AF.Relu
                    )
                else:
                    for j2 in range(2):
                        j = 2 * jj + j2
                        nc.scalar.activation(
                            dst[:, j, cs], ph[:, j2, 0:n], AF.Relu,
                            bias=btile[:, j : j + 1],
                        )

        def emit_stage2(half, pa, pb, mul_eng=None):
            n = pb - pa
            rh2 = hpool.tile([128, HC, n], bf16, tag="rh2", name=f"rh2_{half}")
            mlp(rh2, lambda k: pfT[:, k, pa:pb], slice(0, n), n, t_aw1, t_ab1,
                ab1_zero, f"s2_{half}")
            ps_a = ps_pool.tile([1, n], f32, tag="s", name=f"psa_{half}")
            for j in range(HC):
                nc.tensor.matmul(
                    ps_a[:], t_aw2[:, j : j + 1], rh2[:, j, :],
                    start=(j == 0), stop=(j == HC - 1),
                )
            ea = spool.tile([1, n], f32, tag="ea", name=f"ea_{half}")
            st = spool.tile([1, 1], f32, tag="st", name=f"st_{half}")
            nc.scalar.activation(ea[:], ps_a[:], AF.Exp, accum_out=st[:])
            ebc = xapool.tile([128, n], f32, tag="ebc", name=f"ebc_{half}")
            nc.gpsimd.partition_broadcast(ebc[:], ea[:])
            scr = xapool.tile([128, KC, n], f32, tag="scr", name=f"scr_{half}")
            (mul_eng or nc.vector).tensor_mul(
                scr[:],
                pfT[:, :, pa:pb],
                ebc[:].rearrange("p (x n) -> p x n", x=1).to_broadcast([128, KC, n]),
            )
            part = spool.tile([128, KC], f32, tag="part", name=f"part_{half}")
            nc.vector.reduce_sum(part[:], scr[:], axis=AX.X)
            nc.sync.dma_start(out_part.ap()[half], part[:])
            nc.sync.dma_start(out_stats.ap()[:, half : half + 1], st[:])

        # warm up with two small tail blocks; end with small block nb-3 so the
        # stage-2 dependency chains drain behind cheap PE work. Stage 2 runs
        # in three path ranges: [0, pa) emitted mid-stream (its producers'
        # chains long drained), [pa, pb) after the last stage-1 block, and
        # the small-tail range [pb, PS) last.
        if nb >= 10:
            emit_order = [nb - 1, nb - 2] + list(range(nb - 2))
            a1 = next(i for i in range(nb) if p_offs[i] + blocks[i][0] >= PS // 2)
            pa_split = p_offs[a1] + blocks[a1][0]
            s2a_emit_at = emit_order.index(a1) + 3
            pb_split = p_offs[nb - 3]
        elif nb >= 4:
            emit_order = [nb - 1, nb - 2] + list(range(nb - 2))
            a1 = None
            pa_split = 0
            s2a_emit_at = -1
            pb_split = p_offs[nb - 3]
        else:
            emit_order = [nb - 1] + list(range(nb - 1))
            a1 = None
            pa_split = 0
            s2a_emit_at = -1
            pb_split = 0

        for ei, b in enumerate(emit_order):
            bp, cap = blocks[b]
            rows = rows_list[b]
            x_off, p_off = x_offs[b], p_offs[b]

            x_b = xpool.tile([128, KC, rows], bf16, tag="xb", name=f"xb_{b}")
            nc.sync.dma_start(
                x_b[:],
                xb.ap()[x_off : x_off + KC * 128 * rows].rearrange(
                    "(k d r) -> d k r", k=KC, d=128
                ),
            )

            rh = hpool.tile([128, HC, rows], bf16, tag="rh", name=f"rh_{b}")
            erow = spool.tile([1, rows], bf16, tag="erow", name=f"erow_{b}")
            if rows > 512:
                c0 = ((rows // 2) + 1) // 2 * 2   # even split point
                chunks = [(0, c0), (c0, rows - c0)]
            else:
                chunks = [(0, rows)]
            for (coff, csz) in chunks:
                cs = slice(coff, coff + csz)
                mlp(rh, lambda k: x_b[:, k, cs], cs, csz, t_w1, t_b1, b1_zero,
                    f"b{b}_{coff}")
                ps_s = ps_pool.tile([1, csz], f32, tag="s", name=f"ps_{b}_{coff}")
                for j in range(HC):
                    nc.tensor.matmul(
                        ps_s[:], t_w2[:, j : j + 1], rh[:, j, cs],
                        start=(j == 0), stop=(j == HC - 1),
                    )
                nc.scalar.activation(erow[:, cs], ps_s[:], AF.Exp)

            # xa holds [x*e (KC chunks) | e] so the fold tree computes the
            # weighted sums and the softmax denominator in one pass.
            xa = xapool.tile([128, KC + 1, rows], bf16, tag="xa", name=f"xa_{b}")
            nc.gpsimd.partition_broadcast(xa[:, KC, :], erow[:])
            nc.vector.tensor_mul(
                xa[:, 0:KC, :],
                x_b[:],
                xa[:, KC, :]
                .rearrange("p (x r) -> p x r", x=1)
                .to_broadcast([128, KC, rows]),
            )

            fs = xapool.tile(
                [128, KC + 1, rows // 2], bf16, tag="fold", name=f"fold_{b}"
            )
            cur = xa[:].rearrange("p c (s l) -> p c s l", l=cap)
            l = cap
            toggle = 0
            while l > 1 and l % 2 == 0:
                half = l // 2
                dst_tile = fs if toggle == 0 else xa
                dst = dst_tile[:, :, 0 : bp * half].rearrange(
                    "p c (s l) -> p c s l", l=half
                )
                nc.vector.tensor_add(
                    dst, cur[:, :, :, 0:half], cur[:, :, :, half:l]
                )
                cur = dst
                l = half
                toggle ^= 1

            praw = spool.tile([128, KC + 1, bp], f32, tag="praw", name=f"praw_{b}")
            if l > 1:
                nc.vector.reduce_sum(praw[:], cur, axis=AX.X)
            else:
                nc.vector.tensor_copy(praw[:], cur[:, :, :, 0])
            winv = spool.tile([128, bp], f32, tag="winv", name=f"winv_{b}")
            nc.vector.reciprocal(winv[:], praw[:, KC, :])
            winv_bc = winv[:].rearrange("p (x s) -> p x s", x=1).to_broadcast(
                [128, KC, bp]
            )
            nc.vector.tensor_mul(
                pfT[:, :, p_off : p_off + bp], praw[:, 0:KC, :], winv_bc
            )

            if ei == 0:
                nc.sync.dma_start(t_aw1[:], aw1.ap().rearrange("k d h -> d k h"))
                nc.sync.dma_start(t_ab1[:], ab1.ap())
                nc.sync.dma_start(t_aw2[:], aw2.ap())

            if ei == s2a_emit_at and pa_split > 0:
                emit_stage2(0, 0, pa_split)

        written = {0} if pa_split > 0 else set()
        if pb_split > 0:
            emit_stage2(1, pa_split, pb_split)
            emit_stage2(2, pb_split, PS, mul_eng=nc.gpsimd)
            written |= {1, 2}
        else:
            emit_stage2(1, 0, PS)
            written.add(1)
        unwritten = [z for z in range(3) if z not in written]
        if unwritten:
            zt = spool.tile([128, KC + 1], f32, tag="zt")
            nc.gpsimd.memset(zt[:], 0)
            for z in unwritten:
                nc.sync.dma_start(out_part.ap()[z], zt[:, 0:KC])
                nc.sync.dma_start(out_stats.ap()[:, z : z + 1], zt[0:1, KC : KC + 1])
    nc.compile()
    return nc


def _get_program(blocks, b1_zero, ab1_zero):
    key = (blocks, b1_zero, ab1_zero)
    if key not in _PROG_CACHE:
        _PROG_CACHE[key] = _build_program(blocks, b1_zero, ab1_zero)
    return _PROG_CACHE[key]


def _find_poison(pW1, pb1, pw2):
    """x* (bf16) whose node-MLP score is <= -2e4: exp(score) == 0 exactly."""
    rng = np.random.default_rng(12345)
    v = rng.standard_normal((64, D)).astype(np.float32)
    s_inf = np.maximum(v @ pW1, 0.0) @ pw2        # score slope along t*v
    i = int(np.argmin(s_inf))
    s = float(s_inf[i])
    if s > -0.05:
        s = -0.05
        v[i] *= 0.0
        v[i, 0] = 1.0  # degenerate fallback; never hit for random weights
    t = 30000.0 / (-s)
    xpad = (t * v[i]).astype(ml_dtypes.bfloat16)
    # verify with the exact bf16 values (f32 arithmetic, bias included)
    sc = float(
        np.maximum(xpad.astype(np.float32) @ pW1 + pb1, 0.0) @ pw2
    )
    if sc > -2e4:  # extremely unlikely; rescale using measured slope
        xpad = (xpad.astype(np.float32) * (3e4 / max(-sc, 1.0))).astype(
            ml_dtypes.bfloat16
        )
    return xpad


def _prep(inputs):
    """Host-side sharding/sorting/packing. Returns (blocks, flags, in_maps)."""
    x = np.asarray(inputs["paths_nodes"], dtype=np.float32)
    lengths = np.asarray(inputs["lengths"], dtype=np.int32)
    pW1 = np.asarray(inputs["pW1"], dtype=np.float32)
    pb1 = np.asarray(inputs["pb1"], dtype=np.float32)
    pw2 = np.asarray(inputs["pw2"], dtype=np.float32)
    aW1 = np.asarray(inputs["aW1"], dtype=np.float32)
    ab1 = np.asarray(inputs["ab1"], dtype=np.float32)
    aw2 = np.asarray(inputs["aw2"], dtype=np.float32)
    # pb2 / ab2 shift their softmax logits uniformly -> no effect on output.
    b1_zero = bool(np.all(pb1 == 0.0))
    ab1_zero = bool(np.all(ab1 == 0.0))

    bf = ml_dtypes.bfloat16
    len_sh = lengths.reshape(NCORES, PS)
    orders = np.argsort(-len_sh, axis=1, kind="stable")        # [NC, PS] desc
    sorted_len = np.take_along_axis(len_sh, orders, axis=1)
    len_max = sorted_len.max(axis=0)                           # [PS]
    blocks = _make_blocks(len_max)

    xpad = _find_poison(pW1, pb1, pw2).astype(np.float32)

    x_sh = x.reshape(NCORES, PS, LMAX, D)
    w1_np = np.ascontiguousarray(pW1.reshape(KC, 128, H)).astype(bf)
    w2_np = np.ascontiguousarray(pw2.reshape(HC, 128).T).astype(bf)
    b1_np = np.ascontiguousarray(pb1.reshape(HC, 128).T).astype(np.float32)
    aw1_np = np.ascontiguousarray(aW1.reshape(KC, 128, H)).astype(bf)
    ab1_np = np.ascontiguousarray(ab1.reshape(HC, 128).T).astype(np.float32)
    aw2_np = np.ascontiguousarray(aw2.reshape(HC, 128).T).astype(bf)

    ar = np.arange(LMAX + 2)
    in_maps = []
    for c in range(NCORES):
        xc = x_sh[c][orders[c]]                       # [PS, LMAX, D] sorted
        lc = sorted_len[c]                            # [PS]
        xr_parts = []
        p = 0
        for (bp, cap) in blocks:
            lb = lc[p : p + bp]
            if cap <= LMAX:
                xblk = xc[p : p + bp, :cap, :].copy() # [bp, cap, D]
            else:
                xblk = np.concatenate(
                    [
                        xc[p : p + bp, :, :],
                        np.zeros((bp, cap - LMAX, D), dtype=np.float32),
                    ],
                    axis=1,
                )
            pad = ar[None, :cap] >= lb[:, None]       # [bp, cap]
            xblk[pad] = xpad[None, :]
            xb_t = (
                xblk.reshape(bp, cap, KC, 128)
                .transpose(2, 3, 0, 1)
                .reshape(KC, 128, bp * cap)
            )
            xr_parts.append(xb_t.astype(bf).ravel())
            p += bp
        in_maps.append(
            {
                "xb": np.concatenate(xr_parts),
                "w1": w1_np,
                "w2": w2_np,
                "b1": b1_np,
                "aw1": aw1_np,
                "ab1": ab1_np,
                "aw2": aw2_np,
            }
        )
    return blocks, (b1_zero, ab1_zero), in_maps


def kernel(**inputs):
    global LAST_RESULT
    blocks, flags, in_maps = _prep(inputs)
    nc = _get_program(blocks, *flags)

    res = bass_utils.run_bass_kernel_spmd(
        nc, in_maps, core_ids=list(range(NCORES)), **_TRACE_KW
    )
    LAST_RESULT = res

    parts = np.stack([r["out_part"] for r in res.results])    # [8, 3, 128, KC]
    stats = np.stack([r["out_stats"] for r in res.results])   # [8, 1, 3]
    total = float(stats.sum())
    vec = parts.sum(axis=(0, 1))                              # [128, KC]
    user = np.ascontiguousarray(vec.T).reshape(D) / total
    return user.astype(np.float32)


# revision 20
# speedup vs baseline: 1.0266x; 1.0266x over previous
"""Trainium2 Bass kernel for nn_AttentionNetwork (ragged path attention).

Data-parallel over 8 NeuronCores: 512 paths per core. Paths are sorted by
length (host-side) and packed into variable-width blocks (bp paths x cap
node-slots, bp*cap <= 1024, cap = max length in the block; capacities are
taken as the element-wise max over cores so one SPMD program serves all
8; the last ~64 paths go into <=256-row blocks so the pipeline drains
quickly). All matmuls run in bf16. Padding slots are filled host-side
with a "poison" vector whose MLP score is ~-3e4, so exp underflows to
exactly 0 and no mask machinery is needed on the device. Per block and
~512-column chunk: node-MLP into paired PSUM banks (one fused relu per
bank pair when b1==0, saving ACT instruction overhead) -> score matmuls
-> exp row; the exp row is broadcast across partitions (GpSimd) into
slot KC of a [128, KC+1, rows] tile whose first KC slots get x*e (one
DVE multiply); segment sums (weighted feature sums + the softmax
denominator in slot KC) are computed by a pairwise fold tree while the
segment width stays even (TensorTensor adds beat TensorReduce on the
DVE) with a TensorReduce tail. Stage 2 (path-level attention) is split
into two path ranges interleaved into the tail of stage 1 so the PE
never drains; its softmax skips max-subtraction (logits are O(1)) and
the 8 cores' exp-weighted partial sums are combined on the host.
"""

import sys

if "/opt/trn_rl_repo" not in sys.path:
    sys.path.insert(0, "/opt/trn_rl_repo")

from contextlib import ExitStack

import ml_dtypes
import numpy as np

import concourse.bass as bass  # noqa: F401
import concourse.mybir as mybir
import concourse.tile as tile
from concourse import bacc, bass_utils

P, LMAX, D, H = 4096, 64, 512, 512
NCORES = 8
PS = P // NCORES          # paths per core
KC = D // 128             # contraction chunks
HC = H // 128             # hidden tiles
ROWS_TARGET = 1024        # max rows (bp*cap) per block
TAIL_PATHS = 192          # last paths go into small blocks ...
TAIL_ROWS = 256           # ... of <=256 rows for a fast pipeline drain

f32 = mybir.dt.float32
bf16 = mybir.dt.bfloat16
AF = mybir.ActivationFunctionType
ALU = mybir.AluOpType
AX = mybir.AxisListType

LAST_RESULT = None
_PROG_CACHE = {}
_TRACE_KW = {}


def _make_blocks(len_max):
    """Greedy pack sorted-desc lengths into (bp, cap) blocks.

    cap and bp are kept even so every matmul free dim / path offset is even.
    """
    blocks = []
    i = 0
    while i < PS:
        cap = int(len_max[i])
        if cap % 2:
            cap += 1
        target = TAIL_ROWS if i >= PS - TAIL_PATHS else ROWS_TARGET
        bp = min(max(target // cap, 2), PS - i)
        if bp % 2 and bp > 1:
            bp -= 1
        blocks.append((bp, cap))
        i += bp
    return tuple(blocks)


def _build_program(blocks, b1_zero, ab1_zero):
    """blocks: tuple of (bp, cap); one block = bp paths x cap node slots."""
    nb = len(blocks)
    rows_list = [bp * cap for bp, cap in blocks]
    tot_rows = sum(rows_list)

    nc = bacc.Bacc("TRN2", target_bir_lowering=False, debug=False, num_devices=NCORES)

    xb = nc.dram_tensor("xb", [KC * 128 * tot_rows], bf16, kind="ExternalInput")
    w1 = nc.dram_tensor("w1", [KC, 128, H], bf16, kind="ExternalInput")
    w2 = nc.dram_tensor("w2", [128, HC], bf16, kind="ExternalInput")
    b1 = nc.dram_tensor("b1", [128, HC], f32, kind="ExternalInput")
    aw1 = nc.dram_tensor("aw1", [KC, 128, H], bf16, kind="ExternalInput")
    ab1 = nc.dram_tensor("ab1", [128, HC], f32, kind="ExternalInput")
    aw2 = nc.dram_tensor("aw2", [128, HC], bf16, kind="ExternalInput")
    out_part = nc.dram_tensor("out_part", [3, 128, KC], f32, kind="ExternalOutput")
    out_stats = nc.dram_tensor("out_stats", [1, 3], f32, kind="ExternalOutput")

    x_offs = [0] * nb
    p_offs = [0] * nb
    acc_x = acc_p = 0
    for i in range(nb):
        x_offs[i], p_offs[i] = acc_x, acc_p
        acc_x += KC * 128 * rows_list[i]
        acc_p += blocks[i][0]
    assert acc_p == PS

    with ExitStack() as ctx:
        tc = ctx.enter_context(tile.TileContext(nc))
        const = ctx.enter_context(tc.tile_pool(name="const", bufs=1))
        xpool = ctx.enter_context(tc.tile_pool(name="x", bufs=6))
        xapool = ctx.enter_context(tc.tile_pool(name="xa", bufs=2))
        hpool = ctx.enter_context(tc.tile_pool(name="h", bufs=2))
        spool = ctx.enter_context(tc.tile_pool(name="s", bufs=3))
        ph_pool = ctx.enter_context(tc.tile_pool(name="ph", bufs=3, space="PSUM"))
        ps_pool = ctx.enter_context(tc.tile_pool(name="ps", bufs=2, space="PSUM"))

        t_w1 = const.tile([128, KC, H], bf16)
        nc.sync.dma_start(t_w1[:], w1.ap().rearrange("k d h -> d k h"))
        t_w2 = const.tile([128, HC], bf16)
        nc.sync.dma_start(t_w2[:], w2.ap())
        t_b1 = const.tile([128, HC], f32)
        nc.sync.dma_start(t_b1[:], b1.ap())
        # ACT table prefetch: force the exp table load before data arrives
        t_warm = const.tile([1, 1], f32)
        nc.scalar.activation(t_warm[:], t_b1[0:1, 0:1], AF.Exp)
        t_aw1 = const.tile([128, KC, H], bf16)
        t_ab1 = const.tile([128, HC], f32)
        t_aw2 = const.tile([128, HC], bf16)

        pfT = const.tile([128, KC, PS], bf16)  # normalized path features

        def mlp(dst, rhs_src, cs, n, wtile, btile, bzero, tag):
            """dst[:, j, cs] = relu(W_j.T @ rhs + b_j) for j in 0..HC-1."""
            for jj in range(HC // 2):
                # full-bank tile so each [:, j2, :] slice is bank-aligned
                ph = ph_pool.tile([128, 2, 512], f32, tag="h", name=f"ph_{tag}_{jj}")
                for j2 in range(2):
                    j = 2 * jj + j2
                    for k in range(KC):
                        nc.tensor.matmul(
                            ph[:, j2, 0:n],
                            wtile[:, k, 128 * j : 128 * (j + 1)],
                            rhs_src(k),
                            start=(k == 0),
                            stop=(k == KC - 1),
                        )
                if bzero:
                    nc.scalar.activation(
                        dst[:, 2 * jj : 2 * jj + 2, cs], ph[:, :, 0:n], AF.Relu
                    )
                else:
                    for j2 in range(2):
                        j = 2 * jj + j2
                        nc.scalar.activation(
                            dst[:, j, cs], ph[:, j2, 0:n], AF.Relu,
                            bias=btile[:, j : j + 1],
                        )

        def emit_stage2(half, pa, pb, mul_eng=None):
            n = pb - pa
            rh2 = hpool.tile([128, HC, n], bf16, tag="rh2", name=f"rh2_{half}")
            mlp(rh2, lambda k: pfT[:, k, pa:pb], slice(0, n), n, t_aw1, t_ab1,
                ab1_zero, f"s2_{half}")
            ps_a = ps_pool.tile([1, n], f32, tag="s", name=f"psa_{half}")
            for j in range(HC):
                nc.tensor.matmul(
                    ps_a[:], t_aw2[:, j : j + 1], rh2[:, j, :],
                    start=(j == 0), stop=(j == HC - 1),
                )
            ea = spool.tile([1, n], f32, tag="ea", name=f"ea_{half}")
            st = spool.tile([1, 1], f32, tag="st", name=f"st_{half}")
            nc.scalar.activation(ea[:], ps_a[:], AF.Exp, accum_out=st[:])
            ebc = xapool.tile([128, n], f32, tag="ebc", name=f"ebc_{half}")
            nc.gpsimd.partition_broadcast(ebc[:], ea[:])
            scr = xapool.tile([128, KC, n], f32, tag="scr", name=f"scr_{half}")
            (mul_eng or nc.vector).tensor_mul(
                scr[:],
                pfT[:, :, pa:pb],
                ebc[:].rearrange("p (x n) -> p x n", x=1).to_broadcast([128, KC, n]),
            )
            part = spool.tile([128, KC], f32, tag="part", name=f"part_{half}")
            nc.vector.reduce_sum(part[:], scr[:], axis=AX.X)
            nc.sync.dma_start(out_part.ap()[half], part[:])
            nc.sync.dma_start(out_stats.ap()[:, half : half + 1], st[:])

        # warm up with two small tail blocks; end with small block nb-3 so the
        # stage-2 dependency chains drain behind cheap PE work. Stage 2 runs
        # in three path ranges: [0, pa) emitted mid-stream (its producers'
        # chains long drained), [pa, pb) after the last stage-1 block, and
        # the small-tail range [pb, PS) last.
        if nb >= 10:
            emit_order = [nb - 1, nb - 2] + list(range(nb - 2))
            a1 = next(i for i in range(nb) if p_offs[i] + blocks[i][0] >= PS // 2)
            pa_split = p_offs[a1] + blocks[a1][0]
            s2a_emit_at = emit_order.index(a1) + 3
            pb_split = p_offs[nb - 4]
        elif nb >= 4:
            emit_order = [nb - 1, nb - 2] + list(range(nb - 2))
            a1 = None
            pa_split = 0
            s2a_emit_at = -1
            pb_split = p_offs[nb - 3]
        else:
            emit_order = [nb - 1] + list(range(nb - 1))
            a1 = None
            pa_split = 0
            s2a_emit_at = -1
            pb_split = 0

        for ei, b in enumerate(emit_order):
            bp, cap = blocks[b]
            rows = rows_list[b]
            x_off, p_off = x_offs[b], p_offs[b]

            x_b = xpool.tile([128, KC, rows], bf16, tag="xb", name=f"xb_{b}")
            nc.sync.dma_start(
                x_b[:],
                xb.ap()[x_off : x_off + KC * 128 * rows].rearrange(
                    "(k d r) -> d k r", k=KC, d=128
                ),
            )

            rh = hpool.tile([128, HC, rows], bf16, tag="rh", name=f"rh_{b}")
            erow = spool.tile([1, rows], bf16, tag="erow", name=f"erow_{b}")
            if rows > 512:
                c0 = ((rows // 2) + 1) // 2 * 2   # even split point
                chunks = [(0, c0), (c0, rows - c0)]
            else:
                chunks = [(0, rows)]
            for (coff, csz) in chunks:
                cs = slice(coff, coff + csz)
                mlp(rh, lambda k: x_b[:, k, cs], cs, csz, t_w1, t_b1, b1_zero,
                    f"b{b}_{coff}")
                ps_s = ps_pool.tile([1, csz], f32, tag="s", name=f"ps_{b}_{coff}")
                for j in range(HC):
                    nc.tensor.matmul(
                        ps_s[:], t_w2[:, j : j + 1], rh[:, j, cs],
                        start=(j == 0), stop=(j == HC - 1),
                    )
                nc.scalar.activation(erow[:, cs], ps_s[:], AF.Exp)

            # xa holds [x*e (KC chunks) | e] so the fold tree computes the
            # weighted sums and the softmax denominator in one pass.
            xa = xapool.tile([128, KC + 1, rows], bf16, tag="xa", name=f"xa_{b}")
            nc.gpsimd.partition_broadcast(xa[:, KC, :], erow[:])
            nc.vector.tensor_mul(
                xa[:, 0:KC, :],
                x_b[:],
                xa[:, KC, :]
                .rearrange("p (x r) -> p x r", x=1)
                .to_broadcast([128, KC, rows]),
            )

            fs = xapool.tile(
                [128, KC + 1, rows // 2], bf16, tag="fold", name=f"fold_{b}"
            )
            cur = xa[:].rearrange("p c (s l) -> p c s l", l=cap)
            l = cap
            toggle = 0
            while l > 1 and l % 2 == 0:
                half = l // 2
                dst_tile = fs if toggle == 0 else xa
                dst = dst_tile[:, :, 0 : bp * half].rearrange(
                    "p c (s l) -> p c s l", l=half
                )
                nc.vector.tensor_add(
                    dst, cur[:, :, :, 0:half], cur[:, :, :, half:l]
                )
                cur = dst
                l = half
                toggle ^= 1

            praw = spool.tile([128, KC + 1, bp], f32, tag="praw", name=f"praw_{b}")
            if l > 1:
                nc.vector.reduce_sum(praw[:], cur, axis=AX.X)
            else:
                nc.vector.tensor_copy(praw[:], cur[:, :, :, 0])
            winv = spool.tile([128, bp], f32, tag="winv", name=f"winv_{b}")
            nc.vector.reciprocal(winv[:], praw[:, KC, :])
            winv_bc = winv[:].rearrange("p (x s) -> p x s", x=1).to_broadcast(
                [128, KC, bp]
            )
            nc.vector.tensor_mul(
                pfT[:, :, p_off : p_off + bp], praw[:, 0:KC, :], winv_bc
            )

            if ei == 0:
                nc.sync.dma_start(t_aw1[:], aw1.ap().rearrange("k d h -> d k h"))
                nc.sync.dma_start(t_ab1[:], ab1.ap())
                nc.sync.dma_start(t_aw2[:], aw2.ap())

            if ei == s2a_emit_at and pa_split > 0:
                emit_stage2(0, 0, pa_split)

        written = {0} if pa_split > 0 else set()
        if pb_split > 0:
            emit_stage2(1, pa_split, pb_split)
            emit_stage2(2, pb_split, PS, mul_eng=nc.gpsimd)
            written |= {1, 2}
        else:
            emit_stage2(1, 0, PS)
            written.add(1)
        unwritten = [z for z in range(3) if z not in written]
        if unwritten:
            zt = spool.tile([128, KC + 1], f32, tag="zt")
            nc.gpsimd.memset(zt[:], 0)
            for z in unwritten:
                nc.sync.dma_start(out_part.ap()[z], zt[:, 0:KC])
                nc.sync.dma_start(out_stats.ap()[:, z : z + 1], zt[0:1, KC : KC + 1])
    nc.compile()
    return nc


def _get_program(blocks, b1_zero, ab1_zero):
    key = (blocks, b1_zero, ab1_zero)
    if key not in _PROG_CACHE:
        _PROG_CACHE[key] = _build_program(blocks, b1_zero, ab1_zero)
    return _PROG_CACHE[key]


def _find_poison(pW1, pb1, pw2):
    """x* (bf16) whose node-MLP score is <= -2e4: exp(score) == 0 exactly."""
    rng = np.random.default_rng(12345)
    v = rng.standard_normal((64, D)).astype(np.float32)
    s_inf = np.maximum(v @ pW1, 0.0) @ pw2        # score slope along t*v
    i = int(np.argmin(s_inf))
    s = float(s_inf[i])
    if s > -0.05:
        s = -0.05
        v[i] *= 0.0
        v[i, 0] = 1.0  # degenerate fallback; never hit for random weights
    t = 30000.0 / (-s)
    xpad = (t * v[i]).astype(ml_dtypes.bfloat16)
    # verify with the exact bf16 values (f32 arithmetic, bias included)
    sc = float(
        np.maximum(xpad.astype(np.float32) @ pW1 + pb1, 0.0) @ pw2
    )
    if sc > -2e4:  # extremely unlikely; rescale using measured slope
        xpad = (xpad.astype(np.float32) * (3e4 / max(-sc, 1.0))).astype(
            ml_dtypes.bfloat16
        )
    return xpad


def _prep(inputs):
    """Host-side sharding/sorting/packing. Returns (blocks, flags, in_maps)."""
    x = np.asarray(inputs["paths_nodes"], dtype=np.float32)
    lengths = np.asarray(inputs["lengths"], dtype=np.int32)
    pW1 = np.asarray(inputs["pW1"], dtype=np.float32)
    pb1 = np.asarray(inputs["pb1"], dtype=np.float32)
    pw2 = np.asarray(inputs["pw2"], dtype=np.float32)
    aW1 = np.asarray(inputs["aW1"], dtype=np.float32)
    ab1 = np.asarray(inputs["ab1"], dtype=np.float32)
    aw2 = np.asarray(inputs["aw2"], dtype=np.float32)
    # pb2 / ab2 shift their softmax logits uniformly -> no effect on output.
    b1_zero = bool(np.all(pb1 == 0.0))
    ab1_zero = bool(np.all(ab1 == 0.0))

    bf = ml_dtypes.bfloat16
    len_sh = lengths.reshape(NCORES, PS)
    orders = np.argsort(-len_sh, axis=1, kind="stable")        # [NC, PS] desc
    sorted_len = np.take_along_axis(len_sh, orders, axis=1)
    len_max = sorted_len.max(axis=0)                           # [PS]
    blocks = _make_blocks(len_max)

    xpad = _find_poison(pW1, pb1, pw2).astype(np.float32)

    x_sh = x.reshape(NCORES, PS, LMAX, D)
    w1_np = np.ascontiguousarray(pW1.reshape(KC, 128, H)).astype(bf)
    w2_np = np.ascontiguousarray(pw2.reshape(HC, 128).T).astype(bf)
    b1_np = np.ascontiguousarray(pb1.reshape(HC, 128).T).astype(np.float32)
    aw1_np = np.ascontiguousarray(aW1.reshape(KC, 128, H)).astype(bf)
    ab1_np = np.ascontiguousarray(ab1.reshape(HC, 128).T).astype(np.float32)
    aw2_np = np.ascontiguousarray(aw2.reshape(HC, 128).T).astype(bf)

    ar = np.arange(LMAX + 2)
    in_maps = []
    for c in range(NCORES):
        xc = x_sh[c][orders[c]]                       # [PS, LMAX, D] sorted
        lc = sorted_len[c]                            # [PS]
        xr_parts = []
        p = 0
        for (bp, cap) in blocks:
            lb = lc[p : p + bp]
            if cap <= LMAX:
                xblk = xc[p : p + bp, :cap, :].copy() # [bp, cap, D]
            else:
                xblk = np.concatenate(
                    [
                        xc[p : p + bp, :, :],
                        np.zeros((bp, cap - LMAX, D), dtype=np.float32),
                    ],
                    axis=1,
                )
            pad = ar[None, :cap] >= lb[:, None]       # [bp, cap]
            xblk[pad] = xpad[None, :]
            xb_t = (
                xblk.reshape(bp, cap, KC, 128)
                .transpose(2, 3, 0, 1)
                .reshape(KC, 128, bp * cap)
            )
            xr_parts.append(xb_t.astype(bf).ravel())
            p += bp
        in_maps.append(
            {
                "xb": np.concatenate(xr_parts),
                "w1": w1_np,
                "w2": w2_np,
                "b1": b1_np,
                "aw1": aw1_np,
                "ab1": ab1_np,
                "aw2": aw2_np,
            }
        )
    return blocks, (b1_zero, ab1_zero), in_maps


def kernel(**inputs):
    global LAST_RESULT
    blocks, flags, in_maps = _prep(inputs)
    nc = _get_program(blocks, *flags)

    res = bass_utils.run_bass_kernel_spmd(
        nc, in_maps, core_ids=list(range(NCORES)), **_TRACE_KW
    )
    LAST_RESULT = res

    parts = np.stack([r["out_part"] for r in res.results])    # [8, 3, 128, KC]
    stats = np.stack([r["out_stats"] for r in res.results])   # [8, 1, 3]
    total = float(stats.sum())
    vec = parts.sum(axis=(0, 1))                              # [128, KC]
    user = np.ascontiguousarray(vec.T).reshape(D) / total
    return user.astype(np.float32)


# revision 21
# speedup vs baseline: 1.0782x; 1.0503x over previous
"""Trainium2 Bass kernel for nn_AttentionNetwork (ragged path attention).

Data-parallel over 8 NeuronCores: 512 paths per core. Paths are sorted by
length (host-side) and packed into variable-width blocks (bp paths x cap
node-slots, bp*cap <= 1024, cap = max length in the block; capacities are
taken as the element-wise max over cores so one SPMD program serves all
8; the last ~128 paths go into <=256-row blocks so the pipeline drains
quickly). All matmuls run in bf16. Padding slots are filled host-side
with a "poison" vector whose MLP score is ~-3e4, so exp underflows to
exactly 0 and no mask machinery is needed on the device. Per block and
~512-column chunk: node-MLP into paired PSUM banks (one fused relu per
bank pair when b1==0) -> score matmuls -> exp row; the exp row is
broadcast across partitions (GpSimd) into slot KC of a [128, KC+1, rows]
tile whose first KC slots get x*e (one DVE multiply); the segment sums
(weighted feature sums + softmax denominator in slot KC) are folded
pairwise while the width stays even (TensorTensor adds beat TensorReduce
on the DVE), and the partially-folded tile is DMA'd out in bf16. The
host finishes the last few columns of each segment sum, normalizes, and
runs the tiny stage-2 path-attention (0.2% of the FLOPs) in numpy,
combining all 8 cores' exp-weighted partial sums in one pass. This keeps
the device PE-dense to the last stage-1 block with no cross-engine
drain chains at the end.
"""

import sys

if "/opt/trn_rl_repo" not in sys.path:
    sys.path.insert(0, "/opt/trn_rl_repo")

from contextlib import ExitStack

import ml_dtypes
import numpy as np

import concourse.bass as bass  # noqa: F401
import concourse.mybir as mybir
import concourse.tile as tile
from concourse import bacc, bass_utils

P, LMAX, D, H = 4096, 64, 512, 512
NCORES = 8
PS = P // NCORES          # paths per core
KC = D // 128             # contraction chunks
HC = H // 128             # hidden tiles
ROWS_TARGET = 1024        # max rows (bp*cap) per block
TAIL_PATHS = 128          # last paths go into small blocks ...
TAIL_ROWS = 256           # ... of <=256 rows for a fast pipeline drain

f32 = mybir.dt.float32
bf16 = mybir.dt.bfloat16
AF = mybir.ActivationFunctionType
ALU = mybir.AluOpType
AX = mybir.AxisListType

LAST_RESULT = None
_PROG_CACHE = {}
_TRACE_KW = {}


def _fold_width(cap):
    """Fold halves while even and > 2; the host sums the remaining columns."""
    l = cap
    while l > 2 and l % 2 == 0:
        l //= 2
    return l


def _make_blocks(len_max):
    """Greedy pack sorted-desc lengths into (bp, cap) blocks.

    cap and bp are kept even so every matmul free dim / path offset is even.
    """
    blocks = []
    i = 0
    while i < PS:
        cap = int(len_max[i])
        if cap % 2:
            cap += 1
        target = TAIL_ROWS if i >= PS - TAIL_PATHS else ROWS_TARGET
        bp = min(max(target // cap, 2), PS - i)
        if bp % 2 and bp > 1:
            bp -= 1
        blocks.append((bp, cap))
        i += bp
    return tuple(blocks)


def _build_program(blocks, b1_zero):
    """blocks: tuple of (bp, cap); one block = bp paths x cap node slots."""
    nb = len(blocks)
    rows_list = [bp * cap for bp, cap in blocks]
    tot_rows = sum(rows_list)
    lf_list = [_fold_width(cap) for bp, cap in blocks]
    q_offs = [0] * nb
    acc_q = 0
    for i in range(nb):
        q_offs[i] = acc_q
        acc_q += 128 * (KC + 1) * blocks[i][0] * lf_list[i]

    nc = bacc.Bacc("TRN2", target_bir_lowering=False, debug=False, num_devices=NCORES)

    xb = nc.dram_tensor("xb", [KC * 128 * tot_rows], bf16, kind="ExternalInput")
    w1 = nc.dram_tensor("w1", [KC, 128, H], bf16, kind="ExternalInput")
    w2 = nc.dram_tensor("w2", [128, HC], bf16, kind="ExternalInput")
    b1 = nc.dram_tensor("b1", [128, HC], f32, kind="ExternalInput")
    praw_d = nc.dram_tensor("praw", [acc_q], bf16, kind="ExternalOutput")

    x_offs = [0] * nb
    acc_x = 0
    for i in range(nb):
        x_offs[i] = acc_x
        acc_x += KC * 128 * rows_list[i]

    with ExitStack() as ctx:
        tc = ctx.enter_context(tile.TileContext(nc))
        const = ctx.enter_context(tc.tile_pool(name="const", bufs=1))
        xpool = ctx.enter_context(tc.tile_pool(name="x", bufs=6))
        xapool = ctx.enter_context(tc.tile_pool(name="xa", bufs=2))
        hpool = ctx.enter_context(tc.tile_pool(name="h", bufs=2))
        spool = ctx.enter_context(tc.tile_pool(name="s", bufs=3))
        ph_pool = ctx.enter_context(tc.tile_pool(name="ph", bufs=3, space="PSUM"))
        ps_pool = ctx.enter_context(tc.tile_pool(name="ps", bufs=2, space="PSUM"))

        t_w1 = const.tile([128, KC, H], bf16)
        nc.sync.dma_start(t_w1[:], w1.ap().rearrange("k d h -> d k h"))
        t_w2 = const.tile([128, HC], bf16)
        nc.sync.dma_start(t_w2[:], w2.ap())
        t_b1 = const.tile([128, HC], f32)
        nc.sync.dma_start(t_b1[:], b1.ap())
        # ACT table prefetch: force the exp table load before data arrives
        t_warm = const.tile([1, 1], f32)
        nc.scalar.activation(t_warm[:], t_b1[0:1, 0:1], AF.Exp)

        def mlp(dst, rhs_src, cs, n, wtile, btile, bzero, tag):
            """dst[:, j, cs] = relu(W_j.T @ rhs + b_j) for j in 0..HC-1."""
            for jj in range(HC // 2):
                # full-bank tile so each [:, j2, :] slice is bank-aligned
                ph = ph_pool.tile([128, 2, 512], f32, tag="h", name=f"ph_{tag}_{jj}")
                for j2 in range(2):
                    j = 2 * jj + j2
                    for k in range(KC):
                        nc.tensor.matmul(
                            ph[:, j2, 0:n],
                            wtile[:, k, 128 * j : 128 * (j + 1)],
                            rhs_src(k),
                            start=(k == 0),
                            stop=(k == KC - 1),
                        )
                if bzero:
                    nc.scalar.activation(
                        dst[:, 2 * jj : 2 * jj + 2, cs], ph[:, :, 0:n], AF.Relu
                    )
                else:
                    for j2 in range(2):
                        j = 2 * jj + j2
                        nc.scalar.activation(
                            dst[:, j, cs], ph[:, j2, 0:n], AF.Relu,
                            bias=btile[:, j : j + 1],
                        )

        # warm up with two small tail blocks; end with small tail blocks so
        # the last fold chains drain right behind the final matmuls.
        if nb >= 3:
            emit_order = [nb - 1, nb - 2] + list(range(nb - 2))
        else:
            emit_order = list(range(nb))

        for ei, b in enumerate(emit_order):
            bp, cap = blocks[b]
            rows = rows_list[b]
            x_off = x_offs[b]
            lf = lf_list[b]

            x_b = xpool.tile([128, KC, rows], bf16, tag="xb", name=f"xb_{b}")
            nc.sync.dma_start(
                x_b[:],
                xb.ap()[x_off : x_off + KC * 128 * rows].rearrange(
                    "(k d r) -> d k r", k=KC, d=128
                ),
            )

            rh = hpool.tile([128, HC, rows], bf16, tag="rh", name=f"rh_{b}")
            erow = spool.tile([1, rows], bf16, tag="erow", name=f"erow_{b}")
            if rows > 512:
                c0 = ((rows // 2) + 1) // 2 * 2   # even split point
                chunks = [(0, c0), (c0, rows - c0)]
            else:
                chunks = [(0, rows)]
            for (coff, csz) in chunks:
                cs = slice(coff, coff + csz)
                mlp(rh, lambda k: x_b[:, k, cs], cs, csz, t_w1, t_b1, b1_zero,
                    f"b{b}_{coff}")
                ps_s = ps_pool.tile([1, csz], f32, tag="s", name=f"ps_{b}_{coff}")
                for j in range(HC):
                    nc.tensor.matmul(
                        ps_s[:], t_w2[:, j : j + 1], rh[:, j, cs],
                        start=(j == 0), stop=(j == HC - 1),
                    )
                nc.scalar.activation(erow[:, cs], ps_s[:], AF.Exp)

            # xa holds [x*e (KC chunks) | e] so the fold tree computes the
            # weighted sums and the softmax denominator in one pass.
            xa = xapool.tile([128, KC + 1, rows], bf16, tag="xa", name=f"xa_{b}")
            nc.gpsimd.partition_broadcast(xa[:, KC, :], erow[:])
            nc.vector.tensor_mul(
                xa[:, 0:KC, :],
                x_b[:],
                xa[:, KC, :]
                .rearrange("p (x r) -> p x r", x=1)
                .to_broadcast([128, KC, rows]),
            )

            fs = xapool.tile(
                [128, KC + 1, rows // 2], bf16, tag="fold", name=f"fold_{b}"
            )
            cur = xa[:].rearrange("p c (s l) -> p c s l", l=cap)
            l = cap
            toggle = 0
            while l > 2 and l % 2 == 0:
                half = l // 2
                dst_tile = fs if toggle == 0 else xa
                dst = dst_tile[:, :, 0 : bp * half].rearrange(
                    "p c (s l) -> p c s l", l=half
                )
                nc.vector.tensor_add(
                    dst, cur[:, :, :, 0:half], cur[:, :, :, half:l]
                )
                cur = dst
                l = half
                toggle ^= 1
            assert l == lf

            nc.sync.dma_start(
                praw_d.ap()[
                    q_offs[b] : q_offs[b] + 128 * (KC + 1) * bp * lf
                ].rearrange("(p c s) -> p c s", p=128, c=KC + 1),
                cur.rearrange("p c s l -> p c (s l)"),
            )
    nc.compile()
    return nc


def _get_program(blocks, b1_zero):
    key = (blocks, b1_zero)
    if key not in _PROG_CACHE:
        _PROG_CACHE[key] = _build_program(blocks, b1_zero)
    return _PROG_CACHE[key]


def _find_poison(pW1, pb1, pw2):
    """x* (bf16) whose node-MLP score is <= -2e4: exp(score) == 0 exactly."""
    rng = np.random.default_rng(12345)
    v = rng.standard_normal((64, D)).astype(np.float32)
    s_inf = np.maximum(v @ pW1, 0.0) @ pw2        # score slope along t*v
    i = int(np.argmin(s_inf))
    s = float(s_inf[i])
    if s > -0.05:
        s = -0.05
        v[i] *= 0.0
        v[i, 0] = 1.0  # degenerate fallback; never hit for random weights
    t = 30000.0 / (-s)
    xpad = (t * v[i]).astype(ml_dtypes.bfloat16)
    # verify with the exact bf16 values (f32 arithmetic, bias included)
    sc = float(
        np.maximum(xpad.astype(np.float32) @ pW1 + pb1, 0.0) @ pw2
    )
    if sc > -2e4:  # extremely unlikely; rescale using measured slope
        xpad = (xpad.astype(np.float32) * (3e4 / max(-sc, 1.0))).astype(
            ml_dtypes.bfloat16
        )
    return xpad


def _prep(inputs):
    """Host-side sharding/sorting/packing."""
    x = np.asarray(inputs["paths_nodes"], dtype=np.float32)
    lengths = np.asarray(inputs["lengths"], dtype=np.int32)
    pW1 = np.asarray(inputs["pW1"], dtype=np.float32)
    pb1 = np.asarray(inputs["pb1"], dtype=np.float32)
    pw2 = np.asarray(inputs["pw2"], dtype=np.float32)
    b1_zero = bool(np.all(pb1 == 0.0))

    bf = ml_dtypes.bfloat16
    len_sh = lengths.reshape(NCORES, PS)
    orders = np.argsort(-len_sh, axis=1, kind="stable")        # [NC, PS] desc
    sorted_len = np.take_along_axis(len_sh, orders, axis=1)
    len_max = sorted_len.max(axis=0)                           # [PS]
    blocks = _make_blocks(len_max)

    xpad = _find_poison(pW1, pb1, pw2).astype(np.float32)

    x_sh = x.reshape(NCORES, PS, LMAX, D)
    w1_np = np.ascontiguousarray(pW1.reshape(KC, 128, H)).astype(bf)
    w2_np = np.ascontiguousarray(pw2.reshape(HC, 128).T).astype(bf)
    b1_np = np.ascontiguousarray(pb1.reshape(HC, 128).T).astype(np.float32)

    ar = np.arange(LMAX + 2)
    in_maps = []
    for c in range(NCORES):
        xc = x_sh[c][orders[c]]                       # [PS, LMAX, D] sorted
        lc = sorted_len[c]                            # [PS]
        xr_parts = []
        p = 0
        for (bp, cap) in blocks:
            lb = lc[p : p + bp]
            if cap <= LMAX:
                xblk = xc[p : p + bp, :cap, :].copy() # [bp, cap, D]
            else:
                xblk = np.concatenate(
                    [
                        xc[p : p + bp, :, :],
                        np.zeros((bp, cap - LMAX, D), dtype=np.float32),
                    ],
                    axis=1,
                )
            pad = ar[None, :cap] >= lb[:, None]       # [bp, cap]
            xblk[pad] = xpad[None, :]
            xb_t = (
                xblk.reshape(bp, cap, KC, 128)
                .transpose(2, 3, 0, 1)
                .reshape(KC, 128, bp * cap)
            )
            xr_parts.append(xb_t.astype(bf).ravel())
            p += bp
        in_maps.append(
            {
                "xb": np.concatenate(xr_parts),
                "w1": w1_np,
                "w2": w2_np,
                "b1": b1_np,
            }
        )
    return blocks, b1_zero, in_maps


def kernel(**inputs):
    global LAST_RESULT
    blocks, b1_zero, in_maps = _prep(inputs)
    nc = _get_program(blocks, b1_zero)

    res = bass_utils.run_bass_kernel_spmd(
        nc, in_maps, core_ids=list(range(NCORES)), **_TRACE_KW
    )
    LAST_RESULT = res

    aW1 = np.asarray(inputs["aW1"], dtype=np.float32)
    ab1 = np.asarray(inputs["ab1"], dtype=np.float32)
    aw2 = np.asarray(inputs["aw2"], dtype=np.float32)

    # host: finish segment sums, normalize, stage-2 path attention (tiny)
    nb = len(blocks)
    lf_list = [_fold_width(cap) for bp, cap in blocks]
    pf_all = []
    for c in range(NCORES):
        praw = np.asarray(res.results[c]["praw"], dtype=np.float32)
        q = 0
        pf_core = np.empty((PS, D), dtype=np.float32)
        p = 0
        for i, (bp, cap) in enumerate(blocks):
            lf = lf_list[i]
            seg = praw[q : q + 128 * (KC + 1) * bp * lf].reshape(
                128, KC + 1, bp, lf
            ).sum(axis=3)
            q += 128 * (KC + 1) * bp * lf
            pf = seg[:, 0:KC, :] / seg[:, KC : KC + 1, :]     # [128, KC, bp]
            pf_core[p : p + bp] = pf.transpose(2, 1, 0).reshape(bp, D)
            p += bp
        pf_all.append(pf_core)
    pf_all = np.concatenate(pf_all, axis=0)                   # [P, D] (sorted)

    h2 = np.maximum(pf_all @ aW1 + ab1, 0.0)
    a = h2 @ aw2                                              # [P]
    a -= a.max()
    ea = np.exp(a)
    user = (ea @ pf_all) / ea.sum()
    return user.astype(np.float32)


# revision 23
# speedup vs baseline: 1.1102x; 1.0297x over previous
"""Trainium2 Bass kernel for nn_AttentionNetwork (ragged path attention).

Data-parallel over 8 NeuronCores: 512 paths per core. Paths are sorted by
length (host-side) and packed into variable-width blocks (bp paths x cap
node-slots, bp*cap <= 1024, cap = max length in the block; capacities are
taken as the element-wise max over cores so one SPMD program serves all
8; the last ~128 paths go into <=256-row blocks so the pipeline drains
quickly). All matmuls run in bf16. Padding slots are filled host-side
with a "poison" vector whose MLP score is ~-3e4, so exp underflows to
exactly 0 and no mask machinery is needed on the device. Per block and
~512-column chunk: node-MLP into paired PSUM banks (one fused relu per
bank pair when b1==0) -> score matmuls -> exp row; the exp row is
broadcast across partitions (GpSimd) into slot KC of a [128, KC+1, rows]
tile whose first KC slots get x*e (one DVE multiply); the segment sums
(weighted feature sums + softmax denominator in slot KC) are folded
pairwise while the width stays even (TensorTensor adds beat TensorReduce
on the DVE), and the partially-folded tile is DMA'd out in bf16. The
host finishes the last few columns of each segment sum, normalizes, and
runs the tiny stage-2 path-attention (0.2% of the FLOPs) in numpy,
combining all 8 cores' exp-weighted partial sums in one pass. This keeps
the device PE-dense to the last stage-1 block with no cross-engine
drain chains at the end.
"""

import sys

if "/opt/trn_rl_repo" not in sys.path:
    sys.path.insert(0, "/opt/trn_rl_repo")

from contextlib import ExitStack

import ml_dtypes
import numpy as np

import concourse.bass as bass  # noqa: F401
import concourse.mybir as mybir
import concourse.tile as tile
from concourse import bacc, bass_utils

P, LMAX, D, H = 4096, 64, 512, 512
NCORES = 8
PS = P // NCORES          # paths per core
KC = D // 128             # contraction chunks
HC = H // 128             # hidden tiles
ROWS_TARGET = 1024        # max rows (bp*cap) per block
TAIL_PATHS = 128          # last paths go into small blocks ...
TAIL_ROWS = 256           # ... of <=256 rows for a fast pipeline drain

f32 = mybir.dt.float32
bf16 = mybir.dt.bfloat16
AF = mybir.ActivationFunctionType
ALU = mybir.AluOpType
AX = mybir.AxisListType

LAST_RESULT = None
_PROG_CACHE = {}
_TRACE_KW = {}


def _fold_width(cap):
    """Fold halves while even and > 2; the host sums the remaining columns."""
    l = cap
    while l > 2 and l % 2 == 0:
        l //= 2
    return l


def _make_blocks(len_max):
    """Greedy pack sorted-desc lengths into (bp, cap) blocks.

    cap and bp are kept even so every matmul free dim / path offset is even.
    """
    blocks = []
    i = 0
    while i < PS:
        cap = int(len_max[i])
        if cap % 2:
            cap += 1
        target = TAIL_ROWS if i >= PS - TAIL_PATHS else ROWS_TARGET
        bp = min(max(target // cap, 2), PS - i)
        if bp % 2 and bp > 1:
            bp -= 1
        blocks.append((bp, cap))
        i += bp
    return tuple(blocks)


def _build_program(blocks, b1_zero):
    """blocks: tuple of (bp, cap); one block = bp paths x cap node slots."""
    nb = len(blocks)
    rows_list = [bp * cap for bp, cap in blocks]
    tot_rows = sum(rows_list)
    lf_list = [_fold_width(cap) for bp, cap in blocks]
    q_offs = [0] * nb
    acc_q = 0
    for i in range(nb):
        q_offs[i] = acc_q
        acc_q += 128 * (KC + 1) * blocks[i][0] * lf_list[i]

    nc = bacc.Bacc("TRN2", target_bir_lowering=False, debug=False, num_devices=NCORES)

    xb = nc.dram_tensor("xb", [KC * 128 * tot_rows], bf16, kind="ExternalInput")
    w1 = nc.dram_tensor("w1", [KC, 128, H], bf16, kind="ExternalInput")
    w2 = nc.dram_tensor("w2", [128, HC], bf16, kind="ExternalInput")
    b1 = nc.dram_tensor("b1", [128, HC], f32, kind="ExternalInput")
    praw_d = nc.dram_tensor("praw", [acc_q], bf16, kind="ExternalOutput")

    x_offs = [0] * nb
    acc_x = 0
    for i in range(nb):
        x_offs[i] = acc_x
        acc_x += KC * 128 * rows_list[i]

    with ExitStack() as ctx:
        tc = ctx.enter_context(tile.TileContext(nc))
        const = ctx.enter_context(tc.tile_pool(name="const", bufs=1))
        xpool = ctx.enter_context(tc.tile_pool(name="x", bufs=6))
        xapool = ctx.enter_context(tc.tile_pool(name="xa", bufs=2))
        opool = ctx.enter_context(tc.tile_pool(name="o", bufs=4))
        hpool = ctx.enter_context(tc.tile_pool(name="h", bufs=2))
        spool = ctx.enter_context(tc.tile_pool(name="s", bufs=3))
        ph_pool = ctx.enter_context(tc.tile_pool(name="ph", bufs=3, space="PSUM"))
        ps_pool = ctx.enter_context(tc.tile_pool(name="ps", bufs=2, space="PSUM"))

        t_w1 = const.tile([128, KC, H], bf16)
        nc.sync.dma_start(t_w1[:], w1.ap().rearrange("k d h -> d k h"))
        t_w2 = const.tile([128, HC], bf16)
        nc.sync.dma_start(t_w2[:], w2.ap())
        t_b1 = const.tile([128, HC], f32)
        nc.sync.dma_start(t_b1[:], b1.ap())
        # ACT table prefetch: force the exp table load before data arrives
        t_warm = const.tile([1, 1], f32)
        nc.scalar.activation(t_warm[:], t_b1[0:1, 0:1], AF.Exp)

        def mlp(dst, rhs_src, cs, n, wtile, btile, bzero, tag):
            """dst[:, j, cs] = relu(W_j.T @ rhs + b_j) for j in 0..HC-1."""
            for jj in range(HC // 2):
                # full-bank tile so each [:, j2, :] slice is bank-aligned
                ph = ph_pool.tile([128, 2, 512], f32, tag="h", name=f"ph_{tag}_{jj}")
                for j2 in range(2):
                    j = 2 * jj + j2
                    for k in range(KC):
                        nc.tensor.matmul(
                            ph[:, j2, 0:n],
                            wtile[:, k, 128 * j : 128 * (j + 1)],
                            rhs_src(k),
                            start=(k == 0),
                            stop=(k == KC - 1),
                        )
                if bzero:
                    nc.scalar.activation(
                        dst[:, 2 * jj : 2 * jj + 2, cs], ph[:, :, 0:n], AF.Relu
                    )
                else:
                    for j2 in range(2):
                        j = 2 * jj + j2
                        nc.scalar.activation(
                            dst[:, j, cs], ph[:, j2, 0:n], AF.Relu,
                            bias=btile[:, j : j + 1],
                        )

        # warm up with two small tail blocks; end with small tail blocks so
        # the last fold chains drain right behind the final matmuls.
        if nb >= 3:
            emit_order = [nb - 1, nb - 2] + list(range(nb - 2))
        else:
            emit_order = list(range(nb))

        for ei, b in enumerate(emit_order):
            bp, cap = blocks[b]
            rows = rows_list[b]
            x_off = x_offs[b]
            lf = lf_list[b]

            x_b = xpool.tile([128, KC, rows], bf16, tag="xb", name=f"xb_{b}")
            nc.sync.dma_start(
                x_b[:],
                xb.ap()[x_off : x_off + KC * 128 * rows].rearrange(
                    "(k d r) -> d k r", k=KC, d=128
                ),
            )

            rh = hpool.tile([128, HC, rows], bf16, tag="rh", name=f"rh_{b}")
            erow = spool.tile([1, rows], bf16, tag="erow", name=f"erow_{b}")
            if rows > 512:
                c0 = ((rows // 2) + 1) // 2 * 2   # even split point
                chunks = [(0, c0), (c0, rows - c0)]
            else:
                chunks = [(0, rows)]
            for (coff, csz) in chunks:
                cs = slice(coff, coff + csz)
                mlp(rh, lambda k: x_b[:, k, cs], cs, csz, t_w1, t_b1, b1_zero,
                    f"b{b}_{coff}")
                ps_s = ps_pool.tile([1, csz], f32, tag="s", name=f"ps_{b}_{coff}")
                for j in range(HC):
                    nc.tensor.matmul(
                        ps_s[:], t_w2[:, j : j + 1], rh[:, j, cs],
                        start=(j == 0), stop=(j == HC - 1),
                    )
                nc.scalar.activation(erow[:, cs], ps_s[:], AF.Exp)

            # xa holds [x*e (KC chunks) | e] so the fold tree computes the
            # weighted sums and the softmax denominator in one pass.
            xa = xapool.tile([128, KC + 1, rows], bf16, tag="xa", name=f"xa_{b}")
            nc.gpsimd.partition_broadcast(xa[:, KC, :], erow[:])
            nc.vector.tensor_mul(
                xa[:, 0:KC, :],
                x_b[:],
                xa[:, KC, :]
                .rearrange("p (x r) -> p x r", x=1)
                .to_broadcast([128, KC, rows]),
            )

            # fold into a compact exactly-sized tile at the last level so the
            # praw DMA is one contiguous run per partition (128 descriptors)
            halvings = []
            l = cap
            while l > 2 and l % 2 == 0:
                l //= 2
                halvings.append(l)
            assert l == lf

            cur = xa[:].rearrange("p c (s l) -> p c s l", l=cap)
            out_t = opool.tile(
                [128, KC + 1, bp * lf], bf16, tag="praw", name=f"praw_{b}"
            )
            if halvings:
                fs = xapool.tile(
                    [128, KC + 1, rows // 2], bf16, tag="fold", name=f"fold_{b}"
                )
                toggle = 0
                lc = cap
                for li, half in enumerate(halvings):
                    last = li == len(halvings) - 1
                    if last:
                        dst = out_t[:].rearrange("p c (s l) -> p c s l", l=half)
                    else:
                        dst_tile = fs if toggle == 0 else xa
                        dst = dst_tile[:, :, 0 : bp * half].rearrange(
                            "p c (s l) -> p c s l", l=half
                        )
                    nc.vector.tensor_add(
                        dst, cur[:, :, :, 0:half], cur[:, :, :, half:lc]
                    )
                    cur = dst
                    lc = half
                    toggle ^= 1
                src = out_t[:]
            else:
                src = xa[:].rearrange("p c r -> p (c r)")  # rare: cap == 2
                out_t = None

            nc.sync.dma_start(
                praw_d.ap()[
                    q_offs[b] : q_offs[b] + 128 * (KC + 1) * bp * lf
                ].rearrange("(p cs) -> p cs", p=128),
                src.rearrange("p c s -> p (c s)") if out_t is not None else src,
            )
    nc.compile()
    return nc


def _get_program(blocks, b1_zero):
    key = (blocks, b1_zero)
    if key not in _PROG_CACHE:
        _PROG_CACHE[key] = _build_program(blocks, b1_zero)
    return _PROG_CACHE[key]


def _find_poison(pW1, pb1, pw2):
    """x* (bf16) whose node-MLP score is <= -2e4: exp(score) == 0 exactly."""
    rng = np.random.default_rng(12345)
    v = rng.standard_normal((64, D)).astype(np.float32)
    s_inf = np.maximum(v @ pW1, 0.0) @ pw2        # score slope along t*v
    i = int(np.argmin(s_inf))
    s = float(s_inf[i])
    if s > -0.05:
        s = -0.05
        v[i] *= 0.0
        v[i, 0] = 1.0  # degenerate fallback; never hit for random weights
    t = 30000.0 / (-s)
    xpad = (t * v[i]).astype(ml_dtypes.bfloat16)
    # verify with the exact bf16 values (f32 arithmetic, bias included)
    sc = float(
        np.maximum(xpad.astype(np.float32) @ pW1 + pb1, 0.0) @ pw2
    )
    if sc > -2e4:  # extremely unlikely; rescale using measured slope
        xpad = (xpad.astype(np.float32) * (3e4 / max(-sc, 1.0))).astype(
            ml_dtypes.bfloat16
        )
    return xpad


def _prep(inputs):
    """Host-side sharding/sorting/packing."""
    x = np.asarray(inputs["paths_nodes"], dtype=np.float32)
    lengths = np.asarray(inputs["lengths"], dtype=np.int32)
    pW1 = np.asarray(inputs["pW1"], dtype=np.float32)
    pb1 = np.asarray(inputs["pb1"], dtype=np.float32)
    pw2 = np.asarray(inputs["pw2"], dtype=np.float32)
    b1_zero = bool(np.all(pb1 == 0.0))

    bf = ml_dtypes.bfloat16
    len_sh = lengths.reshape(NCORES, PS)
    orders = np.argsort(-len_sh, axis=1, kind="stable")        # [NC, PS] desc
    sorted_len = np.take_along_axis(len_sh, orders, axis=1)
    len_max = sorted_len.max(axis=0)                           # [PS]
    blocks = _make_blocks(len_max)

    xpad = _find_poison(pW1, pb1, pw2).astype(np.float32)

    x_sh = x.reshape(NCORES, PS, LMAX, D)
    w1_np = np.ascontiguousarray(pW1.reshape(KC, 128, H)).astype(bf)
    w2_np = np.ascontiguousarray(pw2.reshape(HC, 128).T).astype(bf)
    b1_np = np.ascontiguousarray(pb1.reshape(HC, 128).T).astype(np.float32)

    ar = np.arange(LMAX + 2)
    in_maps = []
    for c in range(NCORES):
        xc = x_sh[c][orders[c]]                       # [PS, LMAX, D] sorted
        lc = sorted_len[c]                            # [PS]
        xr_parts = []
        p = 0
        for (bp, cap) in blocks:
            lb = lc[p : p + bp]
            if cap <= LMAX:
                xblk = xc[p : p + bp, :cap, :].copy() # [bp, cap, D]
            else:
                xblk = np.concatenate(
                    [
                        xc[p : p + bp, :, :],
                        np.zeros((bp, cap - LMAX, D), dtype=np.float32),
                    ],
                    axis=1,
                )
            pad = ar[None, :cap] >= lb[:, None]       # [bp, cap]
            xblk[pad] = xpad[None, :]
            xb_t = (
                xblk.reshape(bp, cap, KC, 128)
                .transpose(2, 3, 0, 1)
                .reshape(KC, 128, bp * cap)
            )
            xr_parts.append(xb_t.astype(bf).ravel())
            p += bp
        in_maps.append(
            {
                "xb": np.concatenate(xr_parts),
                "w1": w1_np,
                "w2": w2_np,
                "b1": b1_np,
            }
        )
    return blocks, b1_zero, in_maps


def kernel(**inputs):
    global LAST_RESULT
    blocks, b1_zero, in_maps = _prep(inputs)
    nc = _get_program(blocks, b1_zero)

    res = bass_utils.run_bass_kernel_spmd(
        nc, in_maps, core_ids=list(range(NCORES)), **_TRACE_KW
    )
    LAST_RESULT = res

    aW1 = np.asarray(inputs["aW1"], dtype=np.float32)
    ab1 = np.asarray(inputs["ab1"], dtype=np.float32)
    aw2 = np.asarray(inputs["aw2"], dtype=np.float32)

    # host: finish segment sums, normalize, stage-2 path attention (tiny)
    nb = len(blocks)
    lf_list = [_fold_width(cap) for bp, cap in blocks]
    pf_all = []
    for c in range(NCORES):
        praw = np.asarray(res.results[c]["praw"], dtype=np.float32)
        q = 0
        pf_core = np.empty((PS, D), dtype=np.float32)
        p = 0
        for i, (bp, cap) in enumerate(blocks):
            lf = lf_list[i]
            seg = praw[q : q + 128 * (KC + 1) * bp * lf].reshape(
                128, KC + 1, bp, lf
            ).sum(axis=3)
            q += 128 * (KC + 1) * bp * lf
            pf = seg[:, 0:KC, :] / seg[:, KC : KC + 1, :]     # [128, KC, bp]
            pf_core[p : p + bp] = pf.transpose(2, 1, 0).reshape(bp, D)
            p += bp
        pf_all.append(pf_core)
    pf_all = np.concatenate(pf_all, axis=0)                   # [P, D] (sorted)

    h2 = np.maximum(pf_all @ aW1 + ab1, 0.0)
    a = h2 @ aw2                                              # [P]
    a -= a.max()
    ea = np.exp(a)
    user = (ea @ pf_all) / ea.sum()
    return user.astype(np.float32)
